# revision 3
# baseline (speedup 1.0000x reference)
"""Trainium2 Bass kernel for 3x3 conv (stride 1, pad 1) + bias.

x [32, 64, 224, 224] f32, weight [128, 64, 3, 3] f32, bias [128] f32
-> out [32, 128, 224, 224] f32.

Data-parallel over 8 NeuronCores: core c computes samples [4c, 4c+4).

Per-core scheme (all dims hardcoded):
- conv lowered to 9 accumulated matmuls per 2-output-row block:
  out[oc, 2x224] += w[kh,kw][ic->oc].T @ xpad[ic, rows oh+kh-1, cols kw..kw+223]
- x streamed in 4 row-strips per sample (58 input rows each, 226-wide
  zero-padded rows resident in SBUF), float32r matmuls (1 cycle/row on PE),
  PSUM accumulation, ScalarE evacuation fused with bias add, DMA out.
- weight is transposed/bias reshaped on host (numpy) before upload.
"""
import numpy as np

import concourse.bass as bass
import concourse.mybir as mybir
import concourse.tile as tile
from concourse import bacc
from concourse.bass_utils import run_bass_kernel_spmd
from concourse._compat import axon_active

N_CORES = 8
S = 4                 # samples per core
IC, OC, H, W = 64, 128, 224, 224
WPAD = W + 2          # 226: [lpad, 224 data, rpad]
QROWS = 56            # output rows per strip
SROWS = QROWS + 2     # 58 input-row slots per strip
NQ = H // QROWS       # 4 strips per sample
BLK = 2               # output rows per block
NBLK = QROWS // BLK   # 28 blocks per strip

F32R = mybir.dt.float32r
F32 = mybir.dt.float32


def build_module(repeat=1):
    """Build the per-core Bass module. repeat>1 wraps the compute in a HW loop
    (identical output, used only for marginal timing)."""
    nc = bacc.Bacc("TRN2", target_bir_lowering=False, debug=not axon_active(),
                   enable_asserts=True, num_devices=N_CORES)
    xs = nc.dram_tensor("xs", [S, IC, H, W], F32R, kind="ExternalInput").ap()
    wT = nc.dram_tensor("wT", [IC, 9 * OC], F32R, kind="ExternalInput").ap()
    bias = nc.dram_tensor("bias", [OC, 1], F32, kind="ExternalInput").ap()
    out = nc.dram_tensor("out", [S, OC, H, W], F32, kind="ExternalOutput").ap()

    with tile.TileContext(nc) as tc:
        with tc.tile_pool(name="wp", bufs=1) as wp, \
             tc.tile_pool(name="xp", bufs=2) as xp, \
             tc.tile_pool(name="op", bufs=4) as op, \
             tc.tile_pool(name="pp", bufs=4, space="PSUM") as pp:
            wtile = wp.tile([IC, 9 * OC], F32R)
            btile = wp.tile([OC, 1], F32)
            nc.sync.dma_start(out=wtile, in_=wT)
            nc.sync.dma_start(out=btile, in_=bias)

            # Zero both strip buffers once: pad columns stay zero forever
            # (DMA only ever writes the 224 data columns of each row slot).
            for _ in range(2):
                z = xp.tile([IC, SROWS * WPAD], F32R, tag="strip")
                nc.vector.memset(z.bitcast(F32), 0.0)

            def compute():
                for s in range(S):
                    for q in range(NQ):
                        strip = xp.tile([IC, SROWS * WPAD], F32R, tag="strip")
                        sr = strip.rearrange("p (r c) -> p r c", c=WPAD)
                        r0 = q * QROWS - 1          # input row held by slot 0
                        lo = max(r0, 0)             # first real input row
                        hi = min(r0 + SROWS, H)     # one past last real row
                        s_lo = lo - r0              # slot of first real row
                        s_hi = hi - r0
                        nc.sync.dma_start(
                            out=sr[:, s_lo:s_hi, 1:1 + W],
                            in_=xs[s, :, lo:hi, :],
                        )
                        if q == 0:
                            nc.vector.memset(sr[:, 0, 1:1 + W].bitcast(F32), 0.0)
                        if q == NQ - 1:
                            nc.vector.memset(sr[:, SROWS - 1, 1:1 + W].bitcast(F32), 0.0)

                        for b in range(NBLK):
                            j = b * BLK             # strip-local output row
                            oh = q * QROWS + j      # absolute output row
                            psum = pp.tile([OC, BLK, W], F32)
                            for pos in range(9):
                                kh, kw = divmod(pos, 3)
                                rhs = sr[:, j + kh:j + kh + BLK, kw:kw + W]
                                lhsT = wtile[:, pos * OC:(pos + 1) * OC]
                                nc.tensor.matmul(psum, lhsT, rhs,
                                                 start=(pos == 0), stop=(pos == 8))
                            ot = op.tile([OC, BLK, W], F32)
                            nc.scalar.activation(
                                ot.rearrange("p a b -> p (a b)"),
                                psum.rearrange("p a b -> p (a b)"),
                                mybir.ActivationFunctionType.Identity,
                                bias=btile)
                            nc.sync.dma_start(out=out[s, :, oh:oh + BLK, :], in_=ot)

            if repeat == 1:
                compute()
            else:
                with tc.For_i(0, repeat, 1):
                    compute()

    nc.compile()
    return nc


def host_prep(weight, bias):
    wT = np.ascontiguousarray(
        np.transpose(np.asarray(weight, dtype=np.float32), (1, 2, 3, 0))
    ).reshape(IC, 9 * OC)
    b = np.asarray(bias, dtype=np.float32).reshape(OC, 1)
    return wT, b


_module_cache = {}


def get_module(repeat=1):
    if repeat not in _module_cache:
        _module_cache[repeat] = build_module(repeat)
    return _module_cache[repeat]


def kernel(x, weight, bias):
    x = np.asarray(x, dtype=np.float32)
    wT, b = host_prep(weight, bias)
    nc = get_module()
    in_maps = [{"xs": x[c * S:(c + 1) * S], "wT": wT, "bias": b}
               for c in range(N_CORES)]
    res = run_bass_kernel_spmd(nc, in_maps, core_ids=list(range(N_CORES)))
    return np.concatenate([res.results[c]["out"] for c in range(N_CORES)], axis=0)


# revision 6
# speedup vs baseline: 2.1241x; 2.1241x over previous
"""Trainium2 Bass kernel for 3x3 conv (stride 1, pad 1) + bias.

x [32, 64, 224, 224] f32, weight [128, 64, 3, 3] f32, bias [128] f32
-> out [32, 128, 224, 224] f32.

Data-parallel over 8 NeuronCores: core c computes samples [4c, 4c+4).

Per-core scheme (v2, all dims hardcoded):
- x is zero-padded to [4, 64, 226, 226] on the host, so every strip DMA is
  fully contiguous and all matmul windows are uniform (no edge cases).
- float32r matmuls (1 cycle/row on PE, ~13-bit mantissa).
- K=128 packing: SBUF strip holds padded x rows on partitions 0-63 (top)
  and the same rows shifted one row down on partitions 64-127 (bottom,
  built by one SBUF->SBUF DMA per strip). One K=128 matmul computes the
  kh=0 AND kh=1 contributions together (weights for the two kh stacked on
  the partition halves); kh=2 is a K=64 top-half matmul.
  6 matmuls per 2-output-row block instead of 9.
- Strips of 56 output rows (58 padded input rows), double buffered.
  in/dup DMAs ride the ACT HWDGE ring, store DMAs the SP HWDGE ring, so
  input and output transfers overlap.
- PSUM accumulation; ScalarE evacuates psum->SBUF fused with the bias add;
  store tiles batch 8 output rows so each store DMA moves ~0.9 MB.
- weight is transposed/stacked and bias reshaped on host (numpy).
"""
import numpy as np

import concourse.bass as bass
import concourse.mybir as mybir
import concourse.tile as tile
from concourse import bacc
from concourse.bass_utils import run_bass_kernel_spmd
from concourse._compat import axon_active

N_CORES = 8
S = 4                 # samples per core
IC, OC, H, W = 64, 128, 224, 224
HP, WP = H + 2, W + 2  # padded input dims (226)
QROWS = 56            # output rows per strip
SROWS = QROWS + 2     # 58 padded input rows per strip
NQ = H // QROWS       # 4 strips per sample
BLK = 2               # output rows per block
OBLK = 8              # output rows per store tile (4 blocks)

F32R = mybir.dt.float32r
F32 = mybir.dt.float32


def build_module(repeat=1):
    nc = bacc.Bacc("TRN2", target_bir_lowering=False, debug=not axon_active(),
                   enable_asserts=True, num_devices=N_CORES)
    xs = nc.dram_tensor("xs", [S, IC, HP, WP], F32R, kind="ExternalInput").ap()
    # wpair[0:64, kw*128+oc] = w[oc, ic, kh=0, kw]; [64:128, ...] = kh=1
    wpair = nc.dram_tensor("wpair", [2 * IC, 3 * OC], F32R, kind="ExternalInput").ap()
    # wk2[ic, kw*128+oc] = w[oc, ic, kh=2, kw]
    wk2 = nc.dram_tensor("wk2", [IC, 3 * OC], F32R, kind="ExternalInput").ap()
    bias = nc.dram_tensor("bias", [OC, 1], F32, kind="ExternalInput").ap()
    out = nc.dram_tensor("out", [S, OC, H, W], F32, kind="ExternalOutput").ap()

    with tile.TileContext(nc) as tc:
        with tc.tile_pool(name="wp", bufs=1) as wp, \
             tc.tile_pool(name="xp", bufs=2) as xp, \
             tc.tile_pool(name="op", bufs=3) as op, \
             tc.tile_pool(name="pp", bufs=6, space="PSUM") as pp:
            wpt = wp.tile([2 * IC, 3 * OC], F32R)
            wk2t = wp.tile([IC, 3 * OC], F32R)
            btile = wp.tile([OC, 1], F32)
            nc.sync.dma_start(out=wpt, in_=wpair)
            nc.sync.dma_start(out=wk2t, in_=wk2)
            nc.sync.dma_start(out=btile, in_=bias)

            def compute():
                for s in range(S):
                    for q in range(NQ):
                        # strip covers padded rows 56q .. 56q+58
                        strip = xp.tile([2 * IC, SROWS * WP], F32R, tag="strip")
                        sr = strip.rearrange("p (r c) -> p r c", c=WP)
                        # top half: padded rows, fully contiguous both sides
                        nc.scalar.dma_start(
                            out=sr[0:IC, :, :],
                            in_=xs[s, :, q * QROWS:q * QROWS + SROWS, :])
                        # bottom half = top shifted one row-slot down
                        nc.scalar.dma_start(
                            out=strip[IC:2 * IC, 0:(SROWS - 1) * WP],
                            in_=strip[0:IC, WP:SROWS * WP])

                        for g in range(QROWS // OBLK):
                            ot = op.tile([OC, OBLK, W], F32)
                            for bb in range(OBLK // BLK):
                                u = g * OBLK + bb * BLK
                                oh = q * QROWS + u
                                psum = pp.tile([OC, BLK, W], F32)
                                # slot u holds padded row 56q+u = input row
                                # 56q+u-1; pair mm at slots (u, u+1):
                                #   top    -> rows oh-1, oh   (kh=0)
                                #   bottom -> rows oh,   oh+1 (kh=1)
                                for i, kw in enumerate((0, 1, 2)):
                                    rhs = sr[:, u:u + BLK, kw:kw + W]
                                    nc.tensor.matmul(
                                        psum, wpt[:, kw * OC:(kw + 1) * OC], rhs,
                                        start=(i == 0), stop=False,
                                        skip_group_check=True)
                                # kh=2: input rows oh+1, oh+2 = slots u+2, u+3
                                for i, kw in enumerate((0, 1, 2)):
                                    rhs = sr[0:IC, u + 2:u + 2 + BLK, kw:kw + W]
                                    nc.tensor.matmul(
                                        psum, wk2t[:, kw * OC:(kw + 1) * OC], rhs,
                                        start=False, stop=(i == 2),
                                        skip_group_check=True)
                                nc.scalar.activation(
                                    ot[:, bb * BLK:(bb + 1) * BLK, :].rearrange(
                                        "p a b -> p (a b)"),
                                    psum.rearrange("p a b -> p (a b)"),
                                    mybir.ActivationFunctionType.Identity,
                                    bias=btile)
                            oh0 = q * QROWS + g * OBLK
                            nc.sync.dma_start(out=out[s, :, oh0:oh0 + OBLK, :], in_=ot)

            if repeat == 1:
                compute()
            else:
                with tc.For_i(0, repeat, 1):
                    compute()

    nc.compile()
    return nc


def host_prep(weight, bias):
    w = np.asarray(weight, dtype=np.float32)          # [oc, ic, kh, kw]
    wt = np.transpose(w, (1, 3, 0, 2))                # [ic, kw, oc, kh]
    wpair = np.concatenate([wt[:, :, :, 0], wt[:, :, :, 1]], axis=0) \
        .reshape(2 * IC, 3 * OC)
    wk2 = np.ascontiguousarray(wt[:, :, :, 2]).reshape(IC, 3 * OC)
    b = np.asarray(bias, dtype=np.float32).reshape(OC, 1)
    return wpair, wk2, b


def pad_x(x):
    xp_ = np.zeros((x.shape[0], IC, HP, WP), np.float32)
    xp_[:, :, 1:1 + H, 1:1 + W] = x
    return xp_


_module_cache = {}


def get_module(repeat=1):
    if repeat not in _module_cache:
        _module_cache[repeat] = build_module(repeat)
    return _module_cache[repeat]


def kernel(x, weight, bias):
    x = np.asarray(x, dtype=np.float32)
    wpair, wk2, b = host_prep(weight, bias)
    xp_ = pad_x(x)
    nc = get_module()
    in_maps = [{"xs": xp_[c * S:(c + 1) * S], "wpair": wpair, "wk2": wk2,
                "bias": b} for c in range(N_CORES)]
    res = run_bass_kernel_spmd(nc, in_maps, core_ids=list(range(N_CORES)))
    return np.concatenate([res.results[c]["out"] for c in range(N_CORES)], axis=0)
